# revision 21
# baseline (speedup 1.0000x reference)
"""Trainium2 Bass kernel for Jacobi-KAN layer.

y[b,o] = sum_{i,d} P_d(tanh(x[b,i])) * C[i,o,d],  B=262144, I=O=128, D+1=9,
Jacobi polynomials with a=b=1.

Strategy (pure data parallel over batch, 8 cores):
 - Host re-expresses the degree-8 Jacobi basis in the product basis
   {1, t, w, t*w, w^2, t*w^2, w^3, t*w^3, w^4} with w = 2t^2-1 (all values in
   [-1,1]); folds the 9x9 change of basis into the coefficient tensor in
   float64. The constant plane's contribution is a per-output bias
   c0[o] = sum_i C'[i,o,0], added during the PSUM->SBUF copy, so the PE only
   runs 8 accumulating matmuls per output tile instead of 9.
 - Host pre-transposes each x shard to (128, 32768) fp16 so the contraction
   axis i lands on SBUF partitions with perfectly contiguous DMA.
 - Device, per 2048-col chunk (skewed pipeline, tanh one chunk ahead):
     ACT: t = tanh(x); w2 = Square(2v-1) [fused input affine]; w4 = Square(w2)
     DVE: v = t*t; w = 2v-1 (tensor_scalar); tw; w3 = w*w2; tw2
     GPS: tw3 = t*w3; and per 512-group y = psum + c0 (fp16 out)
 - PE: per 512-column group, 8 accumulating fp16 matmuls with C_d stationary
   (128x128) and basis plane moving (128x512) -> PSUM y^T (o,b) fp32.
 - Host transposes y^T back on gather.
"""

import sys

for _p in ("/opt/trn_rl_repo", "/opt/trn_rl_repo/concourse"):
    if _p not in sys.path:
        sys.path.insert(0, _p)

import numpy as np

import concourse.bacc as bacc
import concourse.bass as bass
import concourse.mybir as mybir
from concourse.bass_utils import run_bass_kernel_spmd
from concourse.tile import TileContext

P = 128
N_CORES = 8
B_TOTAL = 262144
B_CORE = B_TOTAL // N_CORES        # 32768
ND = 9                             # number of basis functions
NMM = 8                            # matmul planes (constant plane folded out)
CHUNK = 2048                       # elementwise chunk (free dim)
NCHUNKS = B_CORE // CHUNK          # 16
GROUP = 512                        # matmul moving free dim
GROUPS_PER_CHUNK = CHUNK // GROUP  # 4

F16 = mybir.dt.float16
F32 = mybir.dt.float32
AF = mybir.ActivationFunctionType
OP = mybir.AluOpType


def _basis_transform():
    """9x9 float64 matrix T with C'[i,o,j] = sum_d C[i,o,d] * T[d,j] such that
    sum_j C'_j * basis_j(t) == sum_d C_d * JacobiP_d(t) for the basis
    [1, t, w, t*w, w^2, t*w^2, w^3, t*w^3, w^4], w = 2t^2-1."""
    import numpy.polynomial.polynomial as NP

    a_, b_ = 1.0, 1.0
    polys = [np.array([1.0]), np.array([0.0, 2.0])]
    for i in range(2, ND):
        Ai = (2 * i + a_ + b_ - 1) * (2 * i + a_ + b_) / (2 * i * (i + a_ + b_))
        Bi = (2 * i + a_ + b_ - 1) * (a_ ** 2 - b_ ** 2) / (
            2 * i * (i + a_ + b_) * (2 * i + a_ + b_ - 2))
        Ci = -2 * (i + a_ - 1) * (i + b_ - 1) * (2 * i + a_ + b_) / (
            2 * i * (i + a_ + b_) * (2 * i + a_ + b_ - 2))
        p = NP.polyadd(NP.polymul([Bi, Ai], polys[i - 1]),
                       NP.polymul([Ci], polys[i - 2]))
        polys.append(p)
    Jm = np.zeros((ND, ND))
    for d, p in enumerate(polys):
        Jm[d, :len(p)] = p

    t = np.array([0.0, 1.0])
    w = np.array([-1.0, 0.0, 2.0])
    w2 = NP.polymul(w, w)
    w3 = NP.polymul(w, w2)
    w4 = NP.polymul(w2, w2)
    basis = [np.array([1.0]), t, w, NP.polymul(t, w), w2, NP.polymul(t, w2),
             w3, NP.polymul(t, w3), w4]
    Bm = np.zeros((ND, ND))
    for j, p in enumerate(basis):
        Bm[j, :len(p)] = p
    return Jm @ np.linalg.inv(Bm)


def _build_module():
    nc = bacc.Bacc(trn_type="TRN2")
    xt = nc.dram_tensor("xt", [P, B_CORE], F16, kind="ExternalInput")
    cw = nc.dram_tensor("cw", [P, NMM * P], F16, kind="ExternalInput")
    yt = nc.dram_tensor("yt", [P, B_CORE], F16, kind="ExternalOutput")

    with TileContext(nc) as tc:
        with (
            tc.tile_pool(name="const", bufs=1) as const_pool,
            tc.tile_pool(name="xin", bufs=3) as xin_pool,
            tc.tile_pool(name="bas", bufs=3) as bas_pool,
            tc.tile_pool(name="yout", bufs=4) as yout_pool,
            tc.tile_pool(name="psum", bufs=4, space="PSUM") as psum_pool,
        ):
            cw_sb = const_pool.tile([P, NMM * P], F16)
            nc.sync.dma_start(cw_sb[:], cw[:, :])
            negone = const_pool.tile([P, 1], F32)
            nc.vector.memset(negone[:], -1.0)

            HALF = CHUNK // 2          # 1024 cols, 2 PSUM banks
            t_tiles = [None] * NCHUNKS
            plane_tiles = [None] * NCHUNKS

            def stage_load(c):
                xin = xin_pool.tile([P, CHUNK], F16, tag="xin")
                nc.sync.dma_start(xin[:], xt[:, c * CHUNK:(c + 1) * CHUNK])
                t = bas_pool.tile([P, CHUNK], F16, tag="t", bufs=4)
                nc.scalar.activation(t[:], xin[:], AF.Tanh)
                t_tiles[c] = t

            def stage_products(c):
                t = t_tiles[c]
                v = bas_pool.tile([P, CHUNK], F16, tag="v")
                w = bas_pool.tile([P, CHUNK], F16, tag="w", bufs=4)
                w2 = bas_pool.tile([P, CHUNK], F16, tag="w2", bufs=4)
                w3 = bas_pool.tile([P, CHUNK], F16, tag="w3", bufs=4)
                w4 = bas_pool.tile([P, CHUNK], F16, tag="w4", bufs=4)
                tw = bas_pool.tile([P, CHUNK], F16, tag="tw", bufs=4)
                tw2 = bas_pool.tile([P, CHUNK], F16, tag="tw2", bufs=4)
                tw3 = bas_pool.tile([P, CHUNK], F16, tag="tw3", bufs=4)

                nc.vector.tensor_tensor(v[:], t[:], t[:], OP.mult)
                nc.scalar.activation(w2[:], v[:], AF.Square,
                                     bias=negone[:, 0:1], scale=2.0)
                nc.vector.tensor_scalar(w[:], v[:], 2.0, -1.0,
                                        OP.mult, OP.add)
                nc.vector.tensor_tensor(tw[:], t[:], w[:], OP.mult)
                nc.vector.tensor_tensor(tw3[:], tw[:], w2[:], OP.mult)
                nc.vector.tensor_tensor(w3[:], w[:], w2[:], OP.mult)
                nc.scalar.activation(w4[:], w2[:], AF.Square)
                nc.vector.tensor_tensor(tw2[:], t[:], w2[:], OP.mult)
                plane_tiles[c] = [t, w, tw, w2, tw2, w3, w4, tw3]

            def stage_matmuls(c):
                planes = plane_tiles[c]
                for h in range(2):
                    acc = psum_pool.tile([P, HALF], F32, tag="acc")
                    for g in range(2):
                        off = h * HALF + g * GROUP
                        loc = g * GROUP
                        for j in range(NMM):
                            nc.tensor.matmul(
                                acc[:, loc:loc + GROUP],
                                cw_sb[:, j * P:(j + 1) * P],
                                planes[j][:, off:off + GROUP],
                                start=(j == 0), stop=(j == NMM - 1))
                    yo = yout_pool.tile([P, HALF], F16, tag="yo")
                    nc.scalar.activation(yo[:], acc[:], AF.Copy)
                    col = c * CHUNK + h * HALF
                    nc.sync.dma_start(yt[:, col:col + HALF], yo[:])

            for c in range(NCHUNKS + 2):
                if c < NCHUNKS:
                    stage_load(c)
                if 1 <= c < NCHUNKS + 1:
                    stage_products(c - 1)
                if c >= 2:
                    stage_matmuls(c - 2)

    # TRN2 allows at most one sync wait per instruction; split multi-wait
    # instructions into event-semaphore chains (normally done in
    # Bacc.compile(), which the bass2jax serialization path does not run).
    from concourse import inst_simplify

    nc.insert_bir_kernel_barrier_sem_inc()
    nc.move_matmul_waits_to_ldweights()
    nc.generate_event_semaphores()
    nc.remove_dead_instructions_after_branch()
    nc.validate_blocks()
    nc.dce_regs()
    nc.thread_jumps()
    nc.remove_dead_blocks()
    nc.remove_dead_allocations()
    nc.verify_switch_hints()
    nc.alloc_regs()
    inst_simplify.simplify(nc)
    nc.fuse_regops()
    nc.fuse_blocks()
    nc.replace_nops_with_events()
    for engine in nc.engines:
        nc.fuse_nops(engine)
    nc.remove_dead_nops()
    nc.remove_dangling_data()
    nc.generate_event_semaphores()
    return nc


_NC_CACHE = None


def _make_in_maps(x: np.ndarray, jacobi_coeffs: np.ndarray) -> list:
    x = np.asarray(x)
    C = np.asarray(jacobi_coeffs)

    T = _basis_transform()
    Cp = np.einsum("iod,dj->ioj", C.astype(np.float64), T)
    # planes for matmul: [t, w, tw, w2, tw2, w3, tw3, w4] = basis idx
    # [1, 2, 3, 4, 5, 6, 7, 8] reordered to match device plane order.
    order = [1, 2, 3, 4, 5, 6, 8, 7]  # t, w, tw, w2, tw2, w3, w4(idx8), tw3(idx7)
    # device planes list: [t, w, tw, w2, tw2, w3, w4, tw3]
    cw = np.ascontiguousarray(
        Cp[:, :, order].transpose(0, 2, 1).reshape(P, NMM * P)
    ).astype(np.float16)
    c0 = Cp[:, :, 0].sum(axis=0).astype(np.float32)  # (O,), added on host

    in_maps = []
    for k in range(N_CORES):
        shard = x[k * B_CORE:(k + 1) * B_CORE].astype(np.float16)
        in_maps.append({
            "xt": np.ascontiguousarray(shard.T),
            "cw": cw,
        })
    return in_maps, c0


def kernel(x: np.ndarray, jacobi_coeffs: np.ndarray) -> np.ndarray:
    global _NC_CACHE
    in_maps, c0 = _make_in_maps(x, jacobi_coeffs)

    if _NC_CACHE is None:
        _NC_CACHE = _build_module()

    res = run_bass_kernel_spmd(_NC_CACHE, in_maps, core_ids=list(range(N_CORES)))
    out = np.concatenate(
        [np.asarray(r["yt"]).astype(np.float32).T for r in res.results], axis=0)
    out += c0[None, :]
    return np.ascontiguousarray(out)


# revision 24
# speedup vs baseline: 1.0572x; 1.0572x over previous
"""Trainium2 Bass kernel for Jacobi-KAN layer.

y[b,o] = sum_{i,d} P_d(tanh(x[b,i])) * C[i,o,d],  B=262144, I=O=128, D+1=9,
Jacobi polynomials with a=b=1.

Strategy (pure data parallel over batch, 8 cores):
 - Host re-expresses the degree-8 Jacobi basis in the product basis
   {1, t, w, t*w, w^2, t*w^2, w^3, t*w^3, w^4} with w = 2t^2-1 (all values in
   [-1,1]); folds the 9x9 change of basis into the coefficient tensor in
   float64. The constant plane's contribution is a per-output bias
   c0[o] = sum_i C'[i,o,0], added during the PSUM->SBUF copy, so the PE only
   runs 8 accumulating matmuls per output tile instead of 9.
 - Host pre-transposes each x shard to (128, 32768) fp16 so the contraction
   axis i lands on SBUF partitions with perfectly contiguous DMA.
 - Device, per 2048-col chunk (skewed pipeline, tanh one chunk ahead):
     ACT: t = tanh(x); w2 = Square(2v-1) [fused input affine]; w4 = Square(w2)
     DVE: v = t*t; w = 2v-1 (tensor_scalar); tw; w3 = w*w2; tw2
     GPS: tw3 = t*w3; and per 512-group y = psum + c0 (fp16 out)
 - PE: per 512-column group, 8 accumulating fp16 matmuls with C_d stationary
   (128x128) and basis plane moving (128x512) -> PSUM y^T (o,b) fp32.
 - Host transposes y^T back on gather.
"""

import sys

for _p in ("/opt/trn_rl_repo", "/opt/trn_rl_repo/concourse"):
    if _p not in sys.path:
        sys.path.insert(0, _p)

import numpy as np

import concourse.bacc as bacc
import concourse.bass as bass
import concourse.mybir as mybir
from concourse.bass_utils import run_bass_kernel_spmd
from concourse.tile import TileContext

P = 128
N_CORES = 8
B_TOTAL = 262144
B_CORE = B_TOTAL // N_CORES        # 32768
ND = 9                             # number of basis functions
NMM = 8                            # matmul planes (constant plane folded out)
CHUNK = 2048                       # elementwise chunk (free dim)
NCHUNKS = B_CORE // CHUNK          # 16
GROUP = 512                        # matmul moving free dim
GROUPS_PER_CHUNK = CHUNK // GROUP  # 4

F16 = mybir.dt.float16
F32 = mybir.dt.float32
AF = mybir.ActivationFunctionType
OP = mybir.AluOpType


def _basis_transform():
    """9x9 float64 matrix T with C'[i,o,j] = sum_d C[i,o,d] * T[d,j] such that
    sum_j C'_j * basis_j(t) == sum_d C_d * JacobiP_d(t) for the basis
    [1, t, w, t*w, w^2, t*w^2, w^3, t*w^3, w^4], w = 2t^2-1."""
    import numpy.polynomial.polynomial as NP

    a_, b_ = 1.0, 1.0
    polys = [np.array([1.0]), np.array([0.0, 2.0])]
    for i in range(2, ND):
        Ai = (2 * i + a_ + b_ - 1) * (2 * i + a_ + b_) / (2 * i * (i + a_ + b_))
        Bi = (2 * i + a_ + b_ - 1) * (a_ ** 2 - b_ ** 2) / (
            2 * i * (i + a_ + b_) * (2 * i + a_ + b_ - 2))
        Ci = -2 * (i + a_ - 1) * (i + b_ - 1) * (2 * i + a_ + b_) / (
            2 * i * (i + a_ + b_) * (2 * i + a_ + b_ - 2))
        p = NP.polyadd(NP.polymul([Bi, Ai], polys[i - 1]),
                       NP.polymul([Ci], polys[i - 2]))
        polys.append(p)
    Jm = np.zeros((ND, ND))
    for d, p in enumerate(polys):
        Jm[d, :len(p)] = p

    t = np.array([0.0, 1.0])
    w = np.array([-1.0, 0.0, 2.0])
    w2 = NP.polymul(w, w)
    w3 = NP.polymul(w, w2)
    w4 = NP.polymul(w2, w2)
    basis = [np.array([1.0]), t, w, NP.polymul(t, w), w2, NP.polymul(t, w2),
             w3, NP.polymul(t, w3), w4]
    Bm = np.zeros((ND, ND))
    for j, p in enumerate(basis):
        Bm[j, :len(p)] = p
    return Jm @ np.linalg.inv(Bm)


def _build_module():
    nc = bacc.Bacc(trn_type="TRN2")
    xt = nc.dram_tensor("xt", [P, B_CORE], F16, kind="ExternalInput")
    cw = nc.dram_tensor("cw", [P, NMM * P], F16, kind="ExternalInput")
    yt = nc.dram_tensor("yt", [P, B_CORE], F16, kind="ExternalOutput")

    with TileContext(nc) as tc:
        with (
            tc.tile_pool(name="const", bufs=1) as const_pool,
            tc.tile_pool(name="xin", bufs=3) as xin_pool,
            tc.tile_pool(name="bas", bufs=3) as bas_pool,
            tc.tile_pool(name="yout", bufs=4) as yout_pool,
            tc.tile_pool(name="psum", bufs=4, space="PSUM") as psum_pool,
        ):
            cw_sb = const_pool.tile([P, NMM * P], F16)
            nc.sync.dma_start(cw_sb[:], cw[:, :])
            negone = const_pool.tile([P, 1], F32)
            nc.vector.memset(negone[:], -1.0)

            HALF = CHUNK // 2          # 1024 cols, 2 PSUM banks
            t_tiles = [None] * NCHUNKS
            plane_tiles = [None] * NCHUNKS

            def stage_load(c):
                xin = xin_pool.tile([P, CHUNK], F16, tag="xin")
                nc.sync.dma_start(xin[:], xt[:, c * CHUNK:(c + 1) * CHUNK])
                t = bas_pool.tile([P, CHUNK], F16, tag="t", bufs=4)
                nc.scalar.activation(t[:], xin[:], AF.Tanh)
                t_tiles[c] = t

            def stage_products(c):
                t = t_tiles[c]
                v = bas_pool.tile([P, CHUNK], F16, tag="v")
                w = bas_pool.tile([P, CHUNK], F16, tag="w", bufs=4)
                w2 = bas_pool.tile([P, CHUNK], F16, tag="w2", bufs=4)
                w3 = bas_pool.tile([P, CHUNK], F16, tag="w3", bufs=4)
                w4a = bas_pool.tile([P, CHUNK // 2], F16, tag="w4a", bufs=4)
                w4b = bas_pool.tile([P, CHUNK // 2], F16, tag="w4b", bufs=4)
                tw = bas_pool.tile([P, CHUNK], F16, tag="tw", bufs=4)
                tw2 = bas_pool.tile([P, CHUNK], F16, tag="tw2", bufs=4)
                tw3 = bas_pool.tile([P, CHUNK], F16, tag="tw3", bufs=4)

                nc.vector.tensor_tensor(v[:], t[:], t[:], OP.mult)
                nc.scalar.activation(w2[:], v[:], AF.Square,
                                     bias=negone[:, 0:1], scale=2.0)
                nc.vector.tensor_scalar(w[:], v[:], 2.0, -1.0,
                                        OP.mult, OP.add)
                nc.vector.tensor_tensor(tw[:], t[:], w[:], OP.mult)
                nc.vector.tensor_tensor(tw3[:], tw[:], w2[:], OP.mult)
                nc.vector.tensor_tensor(w3[:], w[:], w2[:], OP.mult)
                half = CHUNK // 2
                nc.scalar.activation(w4a[:], w2[:, 0:half], AF.Square)
                nc.vector.tensor_tensor(w4b[:], w2[:, half:CHUNK],
                                        w2[:, half:CHUNK], OP.mult)
                nc.vector.tensor_tensor(tw2[:], t[:], w2[:], OP.mult)
                plane_tiles[c] = [t, w, tw, w2, tw2, w3, (w4a, w4b), tw3]

            def stage_matmuls(c):
                planes = plane_tiles[c]
                for h in range(2):
                    acc = psum_pool.tile([P, HALF], F32, tag="acc")
                    for g in range(2):
                        off = h * HALF + g * GROUP
                        loc = g * GROUP
                        for j in range(NMM):
                            pj = planes[j]
                            if isinstance(pj, tuple):
                                rhs = pj[h][:, loc:loc + GROUP]
                            else:
                                rhs = pj[:, off:off + GROUP]
                            nc.tensor.matmul(
                                acc[:, loc:loc + GROUP],
                                cw_sb[:, j * P:(j + 1) * P],
                                rhs,
                                start=(j == 0), stop=(j == NMM - 1))
                    yo = yout_pool.tile([P, HALF], F16, tag="yo")
                    nc.scalar.activation(yo[:], acc[:], AF.Copy)
                    col = c * CHUNK + h * HALF
                    nc.sync.dma_start(yt[:, col:col + HALF], yo[:])

            for c in range(NCHUNKS + 2):
                if c < NCHUNKS:
                    stage_load(c)
                if 1 <= c < NCHUNKS + 1:
                    stage_products(c - 1)
                if c >= 2:
                    stage_matmuls(c - 2)

    # TRN2 allows at most one sync wait per instruction; split multi-wait
    # instructions into event-semaphore chains (normally done in
    # Bacc.compile(), which the bass2jax serialization path does not run).
    from concourse import inst_simplify

    nc.insert_bir_kernel_barrier_sem_inc()
    nc.move_matmul_waits_to_ldweights()
    nc.generate_event_semaphores()
    nc.remove_dead_instructions_after_branch()
    nc.validate_blocks()
    nc.dce_regs()
    nc.thread_jumps()
    nc.remove_dead_blocks()
    nc.remove_dead_allocations()
    nc.verify_switch_hints()
    nc.alloc_regs()
    inst_simplify.simplify(nc)
    nc.fuse_regops()
    nc.fuse_blocks()
    nc.replace_nops_with_events()
    for engine in nc.engines:
        nc.fuse_nops(engine)
    nc.remove_dead_nops()
    nc.remove_dangling_data()
    nc.generate_event_semaphores()
    return nc


_NC_CACHE = None


def _make_in_maps(x: np.ndarray, jacobi_coeffs: np.ndarray) -> list:
    x = np.asarray(x)
    C = np.asarray(jacobi_coeffs)

    T = _basis_transform()
    Cp = np.einsum("iod,dj->ioj", C.astype(np.float64), T)
    # planes for matmul: [t, w, tw, w2, tw2, w3, tw3, w4] = basis idx
    # [1, 2, 3, 4, 5, 6, 7, 8] reordered to match device plane order.
    order = [1, 2, 3, 4, 5, 6, 8, 7]  # t, w, tw, w2, tw2, w3, w4(idx8), tw3(idx7)
    # device planes list: [t, w, tw, w2, tw2, w3, w4, tw3]
    cw = np.ascontiguousarray(
        Cp[:, :, order].transpose(0, 2, 1).reshape(P, NMM * P)
    ).astype(np.float16)
    c0 = Cp[:, :, 0].sum(axis=0).astype(np.float32)  # (O,), added on host

    in_maps = []
    for k in range(N_CORES):
        shard = x[k * B_CORE:(k + 1) * B_CORE].astype(np.float16)
        in_maps.append({
            "xt": np.ascontiguousarray(shard.T),
            "cw": cw,
        })
    return in_maps, c0


def kernel(x: np.ndarray, jacobi_coeffs: np.ndarray) -> np.ndarray:
    global _NC_CACHE
    in_maps, c0 = _make_in_maps(x, jacobi_coeffs)

    if _NC_CACHE is None:
        _NC_CACHE = _build_module()

    res = run_bass_kernel_spmd(_NC_CACHE, in_maps, core_ids=list(range(N_CORES)))
    out = np.concatenate(
        [np.asarray(r["yt"]).astype(np.float32).T for r in res.results], axis=0)
    out += c0[None, :]
    return np.ascontiguousarray(out)


# revision 28
# speedup vs baseline: 1.0843x; 1.0256x over previous
"""Trainium2 Bass kernel for Jacobi-KAN layer.

y[b,o] = sum_{i,d} P_d(tanh(x[b,i])) * C[i,o,d],  B=262144, I=O=128, D+1=9,
Jacobi polynomials with a=b=1.

Strategy (pure data parallel over batch, 8 cores):
 - Host re-expresses the degree-8 Jacobi basis in the product basis
   {1, t, w, t*w, w^2, t*w^2, w^3, t*w^3, w^4} with w = 2t^2-1 (all values in
   [-1,1]); folds the 9x9 change of basis into the coefficient tensor in
   float64. The constant plane's contribution is a per-output bias
   c0[o] = sum_i C'[i,o,0], added during the PSUM->SBUF copy, so the PE only
   runs 8 accumulating matmuls per output tile instead of 9.
 - Host pre-transposes each x shard to (128, 32768) fp16 so the contraction
   axis i lands on SBUF partitions with perfectly contiguous DMA.
 - Device, per 2048-col chunk (skewed pipeline, tanh one chunk ahead):
     ACT: t = tanh(x); w2 = Square(2v-1) [fused input affine]; w4 = Square(w2)
     DVE: v = t*t; w = 2v-1 (tensor_scalar); tw; w3 = w*w2; tw2
     GPS: tw3 = t*w3; and per 512-group y = psum + c0 (fp16 out)
 - PE: per 512-column group, 8 accumulating fp16 matmuls with C_d stationary
   (128x128) and basis plane moving (128x512) -> PSUM y^T (o,b) fp32.
 - Host transposes y^T back on gather.
"""

import sys

for _p in ("/opt/trn_rl_repo", "/opt/trn_rl_repo/concourse"):
    if _p not in sys.path:
        sys.path.insert(0, _p)

import numpy as np

import concourse.bacc as bacc
import concourse.bass as bass
import concourse.mybir as mybir
from concourse.bass_utils import run_bass_kernel_spmd
from concourse.tile import TileContext

P = 128
N_CORES = 8
B_TOTAL = 262144
B_CORE = B_TOTAL // N_CORES        # 32768
ND = 9                             # number of basis functions
NMM = 8                            # matmul planes (constant plane folded out)
CHUNK = 2048                       # elementwise chunk (free dim)
NCHUNKS = B_CORE // CHUNK          # 16
GROUP = 512                        # matmul moving free dim
GROUPS_PER_CHUNK = CHUNK // GROUP  # 4

F16 = mybir.dt.float16
F32 = mybir.dt.float32
AF = mybir.ActivationFunctionType
OP = mybir.AluOpType


def _basis_transform():
    """9x9 float64 matrix T with C'[i,o,j] = sum_d C[i,o,d] * T[d,j] such that
    sum_j C'_j * basis_j(t) == sum_d C_d * JacobiP_d(t) for the basis
    [1, t, w, t*w, w^2, t*w^2, w^3, t*w^3, w^4], w = 2t^2-1."""
    import numpy.polynomial.polynomial as NP

    a_, b_ = 1.0, 1.0
    polys = [np.array([1.0]), np.array([0.0, 2.0])]
    for i in range(2, ND):
        Ai = (2 * i + a_ + b_ - 1) * (2 * i + a_ + b_) / (2 * i * (i + a_ + b_))
        Bi = (2 * i + a_ + b_ - 1) * (a_ ** 2 - b_ ** 2) / (
            2 * i * (i + a_ + b_) * (2 * i + a_ + b_ - 2))
        Ci = -2 * (i + a_ - 1) * (i + b_ - 1) * (2 * i + a_ + b_) / (
            2 * i * (i + a_ + b_) * (2 * i + a_ + b_ - 2))
        p = NP.polyadd(NP.polymul([Bi, Ai], polys[i - 1]),
                       NP.polymul([Ci], polys[i - 2]))
        polys.append(p)
    Jm = np.zeros((ND, ND))
    for d, p in enumerate(polys):
        Jm[d, :len(p)] = p

    t = np.array([0.0, 1.0])
    v = np.array([0.0, 0.0, 1.0])
    w2 = NP.polymul(np.array([-1.0, 0.0, 2.0]), np.array([-1.0, 0.0, 2.0]))
    vw2 = NP.polymul(v, w2)
    w4 = NP.polymul(w2, w2)
    basis = [np.array([1.0]), t, v, NP.polymul(t, v), w2, NP.polymul(t, w2),
             vw2, NP.polymul(t, vw2), w4]
    Bm = np.zeros((ND, ND))
    for j, p in enumerate(basis):
        Bm[j, :len(p)] = p
    return Jm @ np.linalg.inv(Bm)


def _build_module():
    nc = bacc.Bacc(trn_type="TRN2")
    xt = nc.dram_tensor("xt", [P, B_CORE], F16, kind="ExternalInput")
    cw = nc.dram_tensor("cw", [P, NMM * P], F16, kind="ExternalInput")
    yt = nc.dram_tensor("yt", [P, B_CORE], F16, kind="ExternalOutput")

    with TileContext(nc) as tc:
        with (
            tc.tile_pool(name="const", bufs=1) as const_pool,
            tc.tile_pool(name="xin", bufs=3) as xin_pool,
            tc.tile_pool(name="bas", bufs=3) as bas_pool,
            tc.tile_pool(name="yout", bufs=4) as yout_pool,
            tc.tile_pool(name="psum", bufs=4, space="PSUM") as psum_pool,
        ):
            cw_sb = const_pool.tile([P, NMM * P], F16)
            nc.sync.dma_start(cw_sb[:], cw[:, :])
            negone = const_pool.tile([P, 1], F32)
            nc.vector.memset(negone[:], -1.0)

            HALF = CHUNK // 2          # 1024 cols, 2 PSUM banks
            t_tiles = [None] * NCHUNKS
            plane_tiles = [None] * NCHUNKS

            def stage_load(c):
                xin = xin_pool.tile([P, CHUNK], F16, tag="xin")
                nc.sync.dma_start(xin[:], xt[:, c * CHUNK:(c + 1) * CHUNK])
                t = bas_pool.tile([P, CHUNK], F16, tag="t", bufs=4)
                nc.scalar.activation(t[:], xin[:], AF.Tanh)
                t_tiles[c] = t

            def stage_products(c):
                t = t_tiles[c]
                v = bas_pool.tile([P, CHUNK], F16, tag="v", bufs=4)
                w2 = bas_pool.tile([P, CHUNK], F16, tag="w2", bufs=4)
                tv = bas_pool.tile([P, CHUNK], F16, tag="tv", bufs=4)
                vw2 = bas_pool.tile([P, CHUNK], F16, tag="vw2", bufs=4)
                w4a = bas_pool.tile([P, CHUNK // 2], F16, tag="w4a", bufs=4)
                w4b = bas_pool.tile([P, CHUNK // 2], F16, tag="w4b", bufs=4)
                tw2 = bas_pool.tile([P, CHUNK], F16, tag="tw2", bufs=4)
                tvw2 = bas_pool.tile([P, CHUNK], F16, tag="tvw2", bufs=4)

                nc.vector.tensor_tensor(v[:], t[:], t[:], OP.mult)
                nc.scalar.activation(w2[:], v[:], AF.Square,
                                     bias=negone[:, 0:1], scale=2.0)
                nc.vector.tensor_tensor(tv[:], t[:], v[:], OP.mult)
                nc.vector.tensor_tensor(vw2[:], v[:], w2[:], OP.mult)
                half = CHUNK // 2
                nc.scalar.activation(w4a[:], w2[:, 0:half], AF.Square)
                nc.vector.tensor_tensor(w4b[:], w2[:, half:CHUNK],
                                        w2[:, half:CHUNK], OP.mult)
                nc.vector.tensor_tensor(tw2[:], t[:], w2[:], OP.mult)
                nc.vector.tensor_tensor(tvw2[:], tv[:], w2[:], OP.mult)
                plane_tiles[c] = [t, v, tv, w2, tw2, vw2, (w4a, w4b), tvw2]

            def stage_matmuls(c):
                planes = plane_tiles[c]
                for h in range(2):
                    acc = psum_pool.tile([P, HALF], F32, tag="acc")
                    for g in range(2):
                        off = h * HALF + g * GROUP
                        loc = g * GROUP
                        for j in range(NMM):
                            pj = planes[j]
                            if isinstance(pj, tuple):
                                rhs = pj[h][:, loc:loc + GROUP]
                            else:
                                rhs = pj[:, off:off + GROUP]
                            nc.tensor.matmul(
                                acc[:, loc:loc + GROUP],
                                cw_sb[:, j * P:(j + 1) * P],
                                rhs,
                                start=(j == 0), stop=(j == NMM - 1))
                    yo = yout_pool.tile([P, HALF], F16, tag="yo")
                    nc.scalar.activation(yo[:], acc[:], AF.Copy)
                    col = c * CHUNK + h * HALF
                    nc.sync.dma_start(yt[:, col:col + HALF], yo[:])

            for c in range(NCHUNKS + 2):
                if c < NCHUNKS:
                    stage_load(c)
                if 1 <= c < NCHUNKS + 1:
                    stage_products(c - 1)
                if c >= 2:
                    stage_matmuls(c - 2)

    # TRN2 allows at most one sync wait per instruction; split multi-wait
    # instructions into event-semaphore chains (normally done in
    # Bacc.compile(), which the bass2jax serialization path does not run).
    from concourse import inst_simplify

    nc.insert_bir_kernel_barrier_sem_inc()
    nc.move_matmul_waits_to_ldweights()
    nc.generate_event_semaphores()
    nc.remove_dead_instructions_after_branch()
    nc.validate_blocks()
    nc.dce_regs()
    nc.thread_jumps()
    nc.remove_dead_blocks()
    nc.remove_dead_allocations()
    nc.verify_switch_hints()
    nc.alloc_regs()
    inst_simplify.simplify(nc)
    nc.fuse_regops()
    nc.fuse_blocks()
    nc.replace_nops_with_events()
    for engine in nc.engines:
        nc.fuse_nops(engine)
    nc.remove_dead_nops()
    nc.remove_dangling_data()
    nc.generate_event_semaphores()
    return nc


_NC_CACHE = None


def _make_in_maps(x: np.ndarray, jacobi_coeffs: np.ndarray) -> list:
    x = np.asarray(x)
    C = np.asarray(jacobi_coeffs)

    T = _basis_transform()
    Cp = np.einsum("iod,dj->ioj", C.astype(np.float64), T)
    # device plane order [t, v, tv, w2, tw2, vw2, w4, tvw2] = basis idx
    # [1, 2, 3, 4, 5, 6, 8, 7] (basis: 1,t,v,tv,w2,tw2,vw2,tvw2,w4).
    order = [1, 2, 3, 4, 5, 6, 8, 7]
    cw = np.ascontiguousarray(
        Cp[:, :, order].transpose(0, 2, 1).reshape(P, NMM * P)
    ).astype(np.float16)
    c0 = Cp[:, :, 0].sum(axis=0).astype(np.float32)  # (O,), added on host

    in_maps = []
    for k in range(N_CORES):
        shard = x[k * B_CORE:(k + 1) * B_CORE].astype(np.float16)
        in_maps.append({
            "xt": np.ascontiguousarray(shard.T),
            "cw": cw,
        })
    return in_maps, c0


def kernel(x: np.ndarray, jacobi_coeffs: np.ndarray) -> np.ndarray:
    global _NC_CACHE
    in_maps, c0 = _make_in_maps(x, jacobi_coeffs)

    if _NC_CACHE is None:
        _NC_CACHE = _build_module()

    res = run_bass_kernel_spmd(_NC_CACHE, in_maps, core_ids=list(range(N_CORES)))
    out = np.concatenate(
        [np.asarray(r["yt"]).astype(np.float32).T for r in res.results], axis=0)
    out += c0[None, :]
    return np.ascontiguousarray(out)
